# revision 9
# baseline (speedup 1.0000x reference)
"""Trainium2 Bass kernel for MultiHeadLatentAttention (B=2, T=2048, C=2048, 16 heads).

Sharding over 8 NeuronCores: core c = (batch b = c//4, r = c%4).
 - Latent projections (x@wq_a, x@wkv_a) computed token-sharded (quarter r),
   latent-dim on partitions, then AllGather-ed within each 4-core batch
   group as THREE gathers ordered by first need: kv-content (0.75MB),
   kv-pe (0.25MB), q (1MB).  The content half of the kv latent is computed
   first so its gather starts ~25us earlier than a combined gather would.
 - Each core then handles head-group r (4 of 16 heads) for the full
   sequence: up-projections, RoPE+RMSNorm, block-causal attention, and a
   row-shard of the output projection.  Host sums the 4 partials per batch.

All matmuls bf16 with fp32 PSUM accumulation.  RMS/softmax reciprocal
tails run on ScalarE (Rsqrt/Reciprocal activations, output bf16 direct);
per-row broadcasts are selector-matmuls.  All DRAM inputs are host-relaid
partition-contiguous so every load is one big-descriptor DMA.  Score/exp/
PV work on causal-diagonal tiles is column-trimmed.  The output projection
is woven per token-tile into the attention phase.
"""

from contextlib import ExitStack

import numpy as np
import ml_dtypes

import concourse.bass as bass
import concourse.tile as tile
import concourse.mybir as mybir
from concourse import bacc
from concourse.bass_utils import run_bass_kernel_spmd

BF16 = mybir.dt.bfloat16
F32 = mybir.dt.float32
NPBF16 = ml_dtypes.bfloat16
AF = mybir.ActivationFunctionType

P = 128
B, T, C = 2, 2048, 2048
H, D = 16, 128
LORA = 1024
KV_PE = 256           # latent chunks 0-1
CONTENT = 768         # latent chunks 2-7
NC_C = CONTENT // P   # 6
NC_P = KV_PE // P     # 2
EPS = 1.1920929e-07
HG = 4                # heads per core
TQ = 512              # tokens per quarter / query block
NLB = LORA // P       # 8 latent row-blocks
NCC = C // P          # 16 contraction chunks of x
NTT = T // TQ         # 4 token 512-tiles
NKT = T // P          # 16 key tiles of 128
RG = [[0, 1, 2, 3], [4, 5, 6, 7]]

_NC_CACHE = {}


def build_nc():
    nc = bacc.Bacc("TRN2", target_bir_lowering=False, debug=False, num_devices=8)

    # all host-relaid partition-contiguous (see _prepare_in_maps)
    xh = nc.dram_tensor("xh", [P, NCC, TQ], BF16, kind="ExternalInput")
    wkvc = nc.dram_tensor("wkvc", [P, NCC, CONTENT], BF16, kind="ExternalInput")
    wkvp = nc.dram_tensor("wkvp", [P, NCC, KV_PE], BF16, kind="ExternalInput")
    wqa = nc.dram_tensor("wqa", [P, NCC, LORA], BF16, kind="ExternalInput")
    wkb = nc.dram_tensor("wkb", [P, NC_C, HG * D], BF16, kind="ExternalInput")
    wkpe = nc.dram_tensor("wkpe", [P, NC_P, HG * D], BF16, kind="ExternalInput")
    wvb = nc.dram_tensor("wvb", [P, NC_C, HG * D], BF16, kind="ExternalInput")
    wqb = nc.dram_tensor("wqb", [P, NLB, HG * D], BF16, kind="ExternalInput")
    wo = nc.dram_tensor("wo", [P, HG, C], BF16, kind="ExternalInput")
    cos2 = nc.dram_tensor("cos2", [P, T], BF16, kind="ExternalInput")
    sin2n = nc.dram_tensor("sin2n", [P, T], BF16, kind="ExternalInput")
    tri = nc.dram_tensor("tri", [P, P], BF16, kind="ExternalInput")
    outT = nc.dram_tensor("outT", [C, T], BF16, kind="ExternalOutput")

    with tile.TileContext(nc) as tc, ExitStack() as ctx:
        dram = ctx.enter_context(tc.tile_pool(name="dram", bufs=1, space="DRAM"))
        psum = ctx.enter_context(tc.tile_pool(name="psum", bufs=8, space="PSUM"))
        consts = ctx.enter_context(tc.tile_pool(name="consts", bufs=1))
        persist = ctx.enter_context(tc.tile_pool(name="persist", bufs=1))
        tmpsq = ctx.enter_context(tc.tile_pool(name="tmpsq", bufs=4))
        ropep = ctx.enter_context(tc.tile_pool(name="ropep", bufs=6))
        rbfp = ctx.enter_context(tc.tile_pool(name="rbfp", bufs=3))
        rinvp = ctx.enter_context(tc.tile_pool(name="rinvp", bufs=2))
        expool = ctx.enter_context(tc.tile_pool(name="expool", bufs=5))
        accpool = ctx.enter_context(tc.tile_pool(name="accpool", bufs=4))
        castpool = ctx.enter_context(tc.tile_pool(name="castpool", bufs=10))
        opool = ctx.enter_context(tc.tile_pool(name="opool", bufs=3))

        def ps_tile(name):
            return psum.tile([P, TQ], F32, name=name, tag="ps")

        def row_mm(out_tile, h, lhsT, rhs):
            # ones-matmul partition reduction into 32-aligned row 32*h.
            # Each row-MM is its own complete accumulation group (rows are
            # disjoint; a shared group would accumulate onto stale bits).
            tp = (0, 32 * h) if h == 3 else None
            nc.tensor.matmul(out_tile[32 * h:32 * h + 1, :], lhsT, rhs,
                             start=True, stop=True, tile_position=tp)

        # ---- warm-up primer: dep-free dense matmuls so the PE's HAM clock
        # gate is released before the first real matmuls arrive ----
        prime_sb = consts.tile([P, TQ], BF16, name="prime_sb")
        nc.vector.memset(prime_sb[:], 0.001)
        prime_w = consts.tile([P, P], BF16, name="prime_w")
        nc.vector.memset(prime_w[:], 0.001)
        _burst_n = [0]

        def warm_burst(n):
            _burst_n[0] += 1
            bp = ps_tile(f"warm_ps{_burst_n[0]}")
            for i in range(n):
                nc.tensor.matmul(bp[:], prime_w[:], prime_sb[:],
                                 start=(i == 0), stop=(i == n - 1))

        warm_burst(16)

        # ---- constants ----
        cos2_sb = consts.tile([P, T], BF16, name="cos2_sb")
        nc.scalar.dma_start(out=cos2_sb[:], in_=cos2[:])
        sin2n_sb = consts.tile([P, T], BF16, name="sin2n_sb")
        nc.scalar.dma_start(out=sin2n_sb[:], in_=sin2n[:])
        tri_sb = consts.tile([P, P], BF16, name="tri_sb")
        nc.scalar.dma_start(out=tri_sb[:], in_=tri[:])
        ones_red = consts.tile([P, 1], BF16, name="ones_red")
        nc.vector.memset(ones_red[:], 1.0)
        zeros128 = consts.tile([P, 1], F32, name="zeros128")
        nc.vector.memset(zeros128[:], 0.0)
        eps_k128 = consts.tile([P, 1], F32, name="eps_k128")
        nc.vector.memset(eps_k128[:], EPS)
        eps_q128 = consts.tile([P, 1], F32, name="eps_q128")
        nc.vector.memset(eps_q128[:], float(D) * EPS)
        sels = []
        for j in range(4):
            s = consts.tile([P, P], BF16, name=f"sel{j}")
            nc.vector.memset(s[:], 0.0)
            nc.vector.memset(s[32 * j:32 * j + 1, :], 1.0)
            sels.append(s)

        # ---- up-projection + output weights (resident; single big DMAs) ----
        wu = ctx.enter_context(tc.tile_pool(name="wu", bufs=1))
        wkb_sb = wu.tile([P, NC_C, HG * D], BF16, name="wkb_sb")
        nc.scalar.dma_start(out=wkb_sb[:], in_=wkb[:])
        wkpe_sb = wu.tile([P, NC_P, HG * D], BF16, name="wkpe_sb")
        nc.scalar.dma_start(out=wkpe_sb[:], in_=wkpe[:])
        wv_sb = wu.tile([P, NC_C, HG * D], BF16, name="wv_sb")
        nc.scalar.dma_start(out=wv_sb[:], in_=wvb[:])
        wqb_sb = wu.tile([P, NLB, HG * D], BF16, name="wqb_sb")
        nc.scalar.dma_start(out=wqb_sb[:], in_=wqb[:])
        wo_sb = wu.tile([P, HG, C], BF16, name="wo_sb")
        nc.scalar.dma_start(out=wo_sb[:], in_=wo[:])

        # ---- persistent phase products ----
        yTn_sb = persist.tile([P, HG, T], BF16, name="yTn_sb")
        kTn_sb = persist.tile([P, HG, T], BF16, name="kTn_sb")
        qTn_sb = persist.tile([P, HG, T], BF16, name="qTn_sb")
        v_sb = persist.tile([P, NKT, HG * D], BF16, name="v_sb")

        # ---- phase L: latent projections + 3 AllGathers ----
        cc_out = {}

        def latent_pass(wname, wh, ngrp, lwp, lsp):
            """ngrp output col-groups of 128; contraction over 16 x-chunks.
            Copies PSUM->SBUF, DMAs to ccin (gpsimd queue), AllGathers."""
            pss = [ps_tile(f"lat_ps_{wname}{g}") for g in range(ngrp)]
            for cc in range(NCC):
                wt = lwp.tile([P, ngrp * P], BF16, name=f"wt_{wname}{cc}", tag="wt")
                nc.sync.dma_start(out=wt[:], in_=wh[:, cc, :])
                for g in range(ngrp):
                    nc.tensor.matmul(pss[g][:], wt[:, g * P:(g + 1) * P],
                                     xsb[:, cc, :], start=(cc == 0),
                                     stop=(cc == NCC - 1))
            lat = lsp.tile([P, ngrp, TQ], BF16, name=f"lat_{wname}", tag="lat")
            for g in range(ngrp):
                nc.scalar.copy(out=lat[:, g, :], in_=pss[g][:])
            ccin = dram.tile([P, ngrp * TQ], BF16, name=f"cc_in_{wname}",
                             tag=f"cc_in_{wname}")
            ccout = dram.tile([4 * P, ngrp * TQ], BF16, name=f"cc_out_{wname}",
                              tag=f"cc_out_{wname}")
            cc_out[wname] = ccout
            nc.gpsimd.dma_start(out=ccin[:], in_=lat[:])
            nc.gpsimd.collective_compute(
                "AllGather", mybir.AluOpType.bypass, replica_groups=RG,
                ins=[ccin.opt()], outs=[ccout.opt()])

        with tc.tile_pool(name="latw", bufs=4) as lwp, \
             tc.tile_pool(name="latstage", bufs=2) as lsp, \
             tc.tile_pool(name="xpool", bufs=1) as xpool:
            # x quarter: split load (first 4 chunks, then 12) so MMs start early
            xsb = xpool.tile([P, NCC, TQ], BF16, name="xsb")
            nc.sync.dma_start(out=xsb[:, 0:4, :], in_=xh[:, 0:4, :])
            nc.sync.dma_start(out=xsb[:, 4:NCC, :], in_=xh[:, 4:NCC, :])
            latent_pass("kvc", wkvc, NC_C, lwp, lsp)
            latent_pass("kvp", wkvp, NC_P, lwp, lsp)
            latent_pass("q", wqa, NLB, lwp, lsp)

        def load_lat(pool, wname, name, tt, ngrp):
            t = pool.tile([P, ngrp, TQ], BF16, name=name, tag=pool.name)
            nc.sync.dma_start(out=t[:],
                              in_=cc_out[wname][tt * P:(tt + 1) * P, :])
            return t

        # ---- K/V machinery ----
        def kc_pass(tt, kvc_t, kcp):
            # k-content for 4 heads + V for 4 token-128s (content gather only)
            kc_sb = kcp.tile([P, HG, TQ], BF16, name=f"kc_sb{tt}", tag="kc")
            for h in range(HG):
                kc_ps = ps_tile(f"kc_ps_{h}_{tt}")
                for j in range(NC_C):
                    nc.tensor.matmul(kc_ps[:], wkb_sb[:, j, h * D:(h + 1) * D],
                                     kvc_t[:, j, :], start=(j == 0),
                                     stop=(j == NC_C - 1))
                nc.scalar.copy(out=kc_sb[:, h, :], in_=kc_ps[:])
                v_ps = ps_tile(f"v_ps_{tt}_{h}")
                for j in range(NC_C):
                    nc.tensor.matmul(v_ps[:], kvc_t[:, j, h * P:(h + 1) * P],
                                     wv_sb[:, j, :], start=(j == 0),
                                     stop=(j == NC_C - 1))
                nc.scalar.copy(out=v_sb[:, tt * 4 + h, :], in_=v_ps[:])
            return kc_sb

        def kpe_pass(tt, kvp_t, kc_sb):
            # k_pe up-proj + rope + assemble k_un into kTn + sum-of-squares
            ss_k = ps_tile(f"ss_k_{tt}")
            nc.vector.memset(ss_k[:], 1.0)
            sqs = []
            hd = D // 2
            for h in range(HG):
                kpe_ps = ps_tile(f"kpe_ps_{h}_{tt}")
                for j in range(NC_P):
                    nc.tensor.matmul(kpe_ps[:], wkpe_sb[:, j, h * D:(h + 1) * D],
                                     kvp_t[:, j, :], start=(j == 0), stop=(j == 1))
                # kswap = halves of kpe swapped (PSUM reads may cross partitions)
                kswap = ropep.tile([P, TQ], BF16, name=f"kswap_{h}_{tt}", tag="rope")
                nc.scalar.copy(out=kswap[0:hd, :], in_=kpe_ps[hd:D, :])
                nc.scalar.copy(out=kswap[hd:D, :], in_=kpe_ps[0:hd, :])
                t1 = ropep.tile([P, TQ], BF16, name=f"t1_{h}_{tt}", tag="rope")
                nc.vector.tensor_mul(t1[:], kpe_ps[:], cos2_sb[:, tt * TQ:(tt + 1) * TQ])
                t2 = ropep.tile([P, TQ], BF16, name=f"t2_{h}_{tt}", tag="rope")
                nc.vector.tensor_mul(t2[:], kswap[:], sin2n_sb[:, tt * TQ:(tt + 1) * TQ])
                nc.vector.tensor_add(t1[:], t1[:], t2[:])
                k_un = ropep.tile([P, TQ], BF16, name=f"k_un_{h}_{tt}", tag="kun")
                nc.vector.tensor_add(k_un[:], t1[:], kc_sb[:, h, :])
                sq = tmpsq.tile([P, TQ], BF16, name=f"ksq_{h}_{tt}", tag="sq")
                nc.vector.tensor_mul(sq[:], k_un[:], k_un[:])
                sqs.append((k_un, sq))
            for h in range(HG):
                row_mm(ss_k, h, ones_red[:], sqs[h][1][:])
            # rbf = 1/sqrt(ss/D + eps): ScalarE sqrt + fast-approx DVE recip
            rbf = rsqrt_tail(ss_k, 1.0 / D, eps_k128, f"k_{tt}")
            for h in range(HG):
                bc = ps_tile(f"kbc_{h}_{tt}")
                nc.tensor.matmul(bc[:], sels[h][:], rbf[:], start=True, stop=True)
                nc.vector.tensor_mul(kTn_sb[:, h, tt * TQ:(tt + 1) * TQ],
                                     sqs[h][0][:], bc[:])

        # ---- Q sections ----
        def q_sec(tt, qlsb_t):
            ss_q = ps_tile(f"ss_q_{tt}")
            nc.vector.memset(ss_q[:], 1.0)
            qps = []
            for h in range(HG):
                q_ps = ps_tile(f"q_ps_{h}_{tt}")
                qps.append(q_ps)
                for j in range(NLB // 2):
                    nc.tensor.matmul(q_ps[:], wqb_sb[:, j, h * D:(h + 1) * D],
                                     qlsb_t[:, j, :], start=(j == 0), stop=False)
            qcs = []
            sqs = []
            for h in range(HG):
                q_ps = qps[h]
                for j in range(NLB // 2, NLB):
                    nc.tensor.matmul(q_ps[:], wqb_sb[:, j, h * D:(h + 1) * D],
                                     qlsb_t[:, j, :], start=False, stop=(j == NLB - 1))
                qc = castpool.tile([P, TQ], BF16, name=f"qc_{h}_{tt}", tag="cast")
                nc.scalar.copy(out=qc[:], in_=q_ps[:])
                qcs.append(qc)
                sq = tmpsq.tile([P, TQ], BF16, name=f"qsq_{h}_{tt}", tag="sq")
                nc.scalar.activation(sq[:], q_ps[:], AF.Square, bias=zeros128[:], scale=1.0)
                sqs.append(sq)
            for h in range(HG):
                row_mm(ss_q, h, ones_red[:], sqs[h][:])
            # rbf = 1/sqrt(ss + D*eps): folds the 1/sqrt(D) attention scale
            rbf = rsqrt_tail(ss_q, 1.0, eps_q128, f"q_{tt}")
            for h in range(HG):
                bc = ps_tile(f"qbc_{h}_{tt}")
                nc.tensor.matmul(bc[:], sels[h][:], rbf[:], start=True, stop=True)
                nc.vector.tensor_mul(qTn_sb[:, h, tt * TQ:(tt + 1) * TQ],
                                     qcs[h][:], bc[:])

        # ---- attention ----
        def rsqrt_tail(ss, scale, bias, which):
            # bf16 1/sqrt(ss*scale + bias) via ScalarE Sqrt -> DVE approx-recip
            sroot = rinvp.tile([P, TQ], F32, name=f"sroot_{which}", tag="rinv")
            nc.scalar.activation(sroot[:], ss[:], AF.Sqrt, bias=bias[:], scale=scale)
            rinv = rinvp.tile([P, TQ], F32, name=f"rinv_{which}", tag="rinv")
            nc.vector.reciprocal_approx_fast(out=rinv[:], in_=sroot[:])
            rbf = rbfp.tile([P, TQ], BF16, name=f"rbf_{which}", tag="rbf")
            nc.scalar.copy(out=rbf[:], in_=rinv[:])
            return rbf

        def a_tail(qb, den4, ycs):
            rinv = rinvp.tile([P, TQ], F32, name=f"rden_{qb}", tag="rinv")
            nc.vector.reciprocal_approx_fast(out=rinv[:], in_=den4[:])
            rbf = rbfp.tile([P, TQ], BF16, name=f"rdenb_{qb}", tag="rbf")
            nc.scalar.copy(out=rbf[:], in_=rinv[:])
            for h in range(HG):
                bc = ps_tile(f"abc_{h}_{qb}")
                nc.tensor.matmul(bc[:], sels[h][:], rbf[:], start=True, stop=True)
                nc.vector.tensor_mul(yTn_sb[:, h, qb * TQ:(qb + 1) * TQ],
                                     ycs[h][:], bc[:])

        pending_a = []

        def a_block(qb):
            # memset to 1.0 (not 0): unused rows go through reciprocal
            den4 = ps_tile(f"den4_{qb}")
            nc.vector.memset(den4[:], 1.0)
            ycs = []
            nkt = 4 * (qb + 1)
            for h in range(HG):
                yt_ps = ps_tile(f"yt_ps_{h}_{qb}")
                acc = accpool.tile([P, TQ], BF16, name=f"acc_{h}_{qb}", tag="acc")

                def emit_sc(kt):
                    # columns < P*jrel of the diagonal tiles are fully masked:
                    # trim them from the score MM, exp, and PV/acc
                    jrel = kt - 4 * qb
                    c0 = P * jrel if jrel > 0 else 0
                    sc_ps = ps_tile(f"sc_ps_{h}_{qb}_{kt}")
                    nc.tensor.matmul(sc_ps[:, c0:], kTn_sb[:, h, kt * P:(kt + 1) * P],
                                     qTn_sb[:, h, qb * TQ + c0:(qb + 1) * TQ],
                                     start=True, stop=True)
                    ex = expool.tile([P, TQ], BF16, name=f"ex_{h}_{qb}_{kt}", tag="ex")
                    nc.scalar.activation(ex[:, c0:], sc_ps[:, c0:], AF.Exp,
                                         bias=zeros128[:], scale=1.0)
                    if jrel >= 0:
                        if jrel > 0:
                            nc.vector.memset(ex[:, 0:c0], 0.0)
                        nc.vector.tensor_mul(ex[:, c0:c0 + P], ex[:, c0:c0 + P],
                                             tri_sb[:])
                    return (ex, c0)

                def emit_pv(kt, ex, c0):
                    if kt == 0:
                        nc.vector.tensor_copy(out=acc[:], in_=ex[:])
                    else:
                        nc.vector.tensor_add(acc[:, c0:], acc[:, c0:], ex[:, c0:])
                    nc.tensor.matmul(yt_ps[:, c0:], v_sb[:, kt, h * D:(h + 1) * D],
                                     ex[:, c0:], start=(kt == 0),
                                     stop=(kt == nkt - 1))

                # 3-deep lookahead: scores for kt+1..kt+3 are issued before
                # pv(kt) so each pv's exp is ready when the in-order PE
                # reaches it
                exs = {}
                for k0 in range(min(3, nkt)):
                    exs[k0] = emit_sc(k0)
                for kt in range(nkt):
                    if kt + 3 < nkt:
                        exs[kt + 3] = emit_sc(kt + 3)
                    emit_pv(kt, *exs.pop(kt))
                row_mm(den4, h, ones_red[:], acc[:])
                yc = castpool.tile([P, TQ], BF16, name=f"yc_{h}_{qb}", tag="cast")
                nc.scalar.copy(out=yc[:], in_=yt_ps[:])
                ycs.append(yc)
                if pending_a and h == 1:
                    a_tail(*pending_a.pop(0))
            pending_a.append((qb, den4, ycs))

        # ---- output projection, one token-tile ----
        def o_pass(tt):
            for ct in range(C // P):
                o_ps = ps_tile(f"o_ps_{ct}_{tt}")
                for h in range(HG):
                    nc.tensor.matmul(o_ps[:], wo_sb[:, h, ct * P:(ct + 1) * P],
                                     yTn_sb[:, h, tt * TQ:(tt + 1) * TQ],
                                     start=(h == 0), stop=(h == HG - 1))
                o_sb = opool.tile([P, TQ], BF16, name=f"o_sb_{ct}_{tt}", tag="o_sb")
                nc.vector.tensor_copy(out=o_sb[:], in_=o_ps[:])
                nc.sync.dma_start(out=outT[ct * P:(ct + 1) * P, tt * TQ:(tt + 1) * TQ],
                                  in_=o_sb[:])

        # ---- emission: K/V content pass, pe pass, then Q woven with A/O ----
        with tc.tile_pool(name="kvcpool", bufs=2) as kvcpool, \
             tc.tile_pool(name="kvppool", bufs=2) as kvppool, \
             tc.tile_pool(name="kcsbp", bufs=2) as kcsbp, \
             tc.tile_pool(name="qlpool", bufs=2) as qlpool:
            warm_burst(24)
            # kc0,kc1,pe0,kc2,pe1,kc3,pe2,pe3: pe(tt) emitted one content
            # pass after kc(tt) so the pe gather has landed by then and only
            # two kc_sb tiles are live at once
            kc_sbs = {}
            for tt in range(NTT):
                kvc_t = load_lat(kvcpool, "kvc", f"kvcsb{tt}", tt, NC_C)
                kc_sbs[tt] = kc_pass(tt, kvc_t, kcsbp)
                if tt >= 1:
                    kvp_t = load_lat(kvppool, "kvp", f"kvpsb{tt-1}", tt - 1, NC_P)
                    kpe_pass(tt - 1, kvp_t, kc_sbs.pop(tt - 1))
            kvp_t = load_lat(kvppool, "kvp", f"kvpsb{NTT-1}", NTT - 1, NC_P)
            kpe_pass(NTT - 1, kvp_t, kc_sbs.pop(NTT - 1))

            warm_burst(24)
            pending_q = []
            for tt in range(NTT):
                qlsb_t = load_lat(qlpool, "q", f"qlsb{tt}", tt, NLB)
                q_sec(tt, qlsb_t)
                if pending_q:
                    qb = pending_q.pop(0)
                    a_block(qb)
                    if qb >= 1:
                        o_pass(qb - 1)
                pending_q.append(tt)
            qb = pending_q.pop(0)
            a_block(qb)
            o_pass(qb - 1)
            a_tail(*pending_a.pop(0))
            o_pass(NTT - 1)

    nc.compile()
    return nc


def _get_nc():
    if "nc" not in _NC_CACHE:
        _NC_CACHE["nc"] = build_nc()
    return _NC_CACHE["nc"]


def _prepare_in_maps(x, cos, sin, wq_a, wq_b, wkv_a, wk_b, wkpe_b, wv_b, wo):
    def bf(a):
        return np.ascontiguousarray(a).astype(NPBF16)

    cosT = np.asarray(cos, np.float32)[0, :, 0, :].T   # (64, T)
    sinT = np.asarray(sin, np.float32)[0, :, 0, :].T
    cos2 = bf(np.concatenate([cosT, cosT], axis=0))    # (128, T)
    sin2n = bf(np.concatenate([sinT, -sinT], axis=0))
    tri = (np.arange(P)[:, None] <= np.arange(P)[None, :]).astype(NPBF16)

    x = np.asarray(x, np.float32)
    wkv_a = np.asarray(wkv_a, np.float32)
    wq_a = np.asarray(wq_a, np.float32)
    # (j p) m -> p j m relayouts (partition-contiguous DRAM)
    wkvc_h = bf(wkv_a.reshape(NCC, P, LORA)[:, :, KV_PE:].transpose(1, 0, 2))
    wkvp_h = bf(wkv_a.reshape(NCC, P, LORA)[:, :, :KV_PE].transpose(1, 0, 2))
    wqa_h = bf(wq_a.reshape(NCC, P, LORA).transpose(1, 0, 2))

    def up_relay(w, k):
        return bf(np.asarray(w, np.float32).reshape(k, P, -1).transpose(1, 0, 2))

    in_maps = []
    for c in range(8):
        b, r = c // 4, c % 4
        xq = x[b, r * TQ:(r + 1) * TQ, :]                  # (512, 2048)
        xh = bf(xq.reshape(TQ, NCC, P).transpose(2, 1, 0))  # (128, 16, 512)
        hgs = slice(r * HG * D, (r + 1) * HG * D)
        wo_c = np.asarray(wo, np.float32)[hgs, :]          # (512, 2048)
        wo_h = bf(wo_c.reshape(HG, P, C).transpose(1, 0, 2))
        in_maps.append({
            "xh": xh,
            "wkvc": wkvc_h,
            "wkvp": wkvp_h,
            "wqa": wqa_h,
            "wkb": up_relay(np.asarray(wk_b, np.float32)[:, hgs], NC_C),
            "wkpe": up_relay(np.asarray(wkpe_b, np.float32)[:, hgs], NC_P),
            "wvb": up_relay(np.asarray(wv_b, np.float32)[:, hgs], NC_C),
            "wqb": up_relay(np.asarray(wq_b, np.float32)[:, hgs], NLB),
            "wo": wo_h,
            "cos2": cos2,
            "sin2n": sin2n,
            "tri": tri,
        })
    return in_maps


def _assemble(results):
    out = np.empty((B, T, C), np.float32)
    for b in range(B):
        acc = results[4 * b]["outT"].astype(np.float32)
        for r in range(1, 4):
            acc = acc + results[4 * b + r]["outT"].astype(np.float32)
        out[b] = acc.T
    return out


def _run(inputs, trace=False):
    nc = _get_nc()
    in_maps = _prepare_in_maps(**inputs)
    res = run_bass_kernel_spmd(nc, in_maps, core_ids=list(range(8)), trace=trace)
    return _assemble(res.results), res


def kernel(**inputs):
    out, _ = _run(inputs)
    return out


# revision 12
# speedup vs baseline: 1.0441x; 1.0441x over previous
"""Trainium2 Bass kernel for MultiHeadLatentAttention (B=2, T=2048, C=2048, 16 heads).

Sharding over 8 NeuronCores: core c = (batch b = c//4, r = c%4).
 - Latent projections (x@wq_a, x@wkv_a) computed token-sharded (quarter r),
   latent-dim on partitions, then AllGather-ed within each 4-core batch
   group as THREE gathers ordered by first need: kv-content (0.75MB),
   kv-pe (0.25MB), q (1MB).  The content half of the kv latent is computed
   first so its gather starts ~25us earlier than a combined gather would.
 - Each core then handles head-group r (4 of 16 heads) for the full
   sequence: up-projections, RoPE+RMSNorm, block-causal attention, and a
   row-shard of the output projection.  Host sums the 4 partials per batch.

All matmuls bf16 with fp32 PSUM accumulation.  RMS/softmax reciprocal
tails run on ScalarE (Rsqrt/Reciprocal activations, output bf16 direct);
per-row broadcasts are selector-matmuls.  All DRAM inputs are host-relaid
partition-contiguous so every load is one big-descriptor DMA.  Score/exp/
PV work on causal-diagonal tiles is column-trimmed.  The output projection
is woven per token-tile into the attention phase.
"""

from contextlib import ExitStack

import numpy as np
import ml_dtypes

import concourse.bass as bass
import concourse.tile as tile
import concourse.mybir as mybir
from concourse import bacc
from concourse.bass_utils import run_bass_kernel_spmd

BF16 = mybir.dt.bfloat16
F32 = mybir.dt.float32
NPBF16 = ml_dtypes.bfloat16
AF = mybir.ActivationFunctionType

P = 128
B, T, C = 2, 2048, 2048
H, D = 16, 128
LORA = 1024
KV_PE = 256           # latent chunks 0-1
CONTENT = 768         # latent chunks 2-7
NC_C = CONTENT // P   # 6
NC_P = KV_PE // P     # 2
EPS = 1.1920929e-07
HG = 4                # heads per core
TQ = 512              # tokens per quarter / query block
NLB = LORA // P       # 8 latent row-blocks
NCC = C // P          # 16 contraction chunks of x
NTT = T // TQ         # 4 token 512-tiles
NKT = T // P          # 16 key tiles of 128
RG = [[0, 1, 2, 3], [4, 5, 6, 7]]

_NC_CACHE = {}


def build_nc():
    nc = bacc.Bacc("TRN2", target_bir_lowering=False, debug=False, num_devices=8)

    # all host-relaid partition-contiguous (see _prepare_in_maps)
    xh = nc.dram_tensor("xh", [P, NCC, TQ], BF16, kind="ExternalInput")
    wkvc = nc.dram_tensor("wkvc", [P, NCC, CONTENT], BF16, kind="ExternalInput")
    wkvp = nc.dram_tensor("wkvp", [P, NCC, KV_PE], BF16, kind="ExternalInput")
    wqa = nc.dram_tensor("wqa", [P, NCC, LORA], BF16, kind="ExternalInput")
    wkb = nc.dram_tensor("wkb", [P, NC_C, HG * D], BF16, kind="ExternalInput")
    wkpe = nc.dram_tensor("wkpe", [P, NC_P, HG * D], BF16, kind="ExternalInput")
    wvb = nc.dram_tensor("wvb", [P, NC_C, HG * D], BF16, kind="ExternalInput")
    wqb = nc.dram_tensor("wqb", [P, NLB, HG * D], BF16, kind="ExternalInput")
    wo = nc.dram_tensor("wo", [P, HG, C], BF16, kind="ExternalInput")
    cos2 = nc.dram_tensor("cos2", [P, T], BF16, kind="ExternalInput")
    sin2n = nc.dram_tensor("sin2n", [P, T], BF16, kind="ExternalInput")
    tri = nc.dram_tensor("tri", [P, P], BF16, kind="ExternalInput")
    outT = nc.dram_tensor("outT", [C, T], BF16, kind="ExternalOutput")

    with tile.TileContext(nc) as tc, ExitStack() as ctx:
        dram = ctx.enter_context(tc.tile_pool(name="dram", bufs=1, space="DRAM"))
        psum = ctx.enter_context(tc.tile_pool(name="psum", bufs=8, space="PSUM"))
        consts = ctx.enter_context(tc.tile_pool(name="consts", bufs=1))
        persist = ctx.enter_context(tc.tile_pool(name="persist", bufs=1))
        tmpsq = ctx.enter_context(tc.tile_pool(name="tmpsq", bufs=8))
        ropep = ctx.enter_context(tc.tile_pool(name="ropep", bufs=4))
        kunp = ctx.enter_context(tc.tile_pool(name="kunp", bufs=8))
        rbfp = ctx.enter_context(tc.tile_pool(name="rbfp", bufs=2))
        rinvp = ctx.enter_context(tc.tile_pool(name="rinvp", bufs=2))
        expool = ctx.enter_context(tc.tile_pool(name="expool", bufs=4))
        accpool = ctx.enter_context(tc.tile_pool(name="accpool", bufs=4))
        castpool = ctx.enter_context(tc.tile_pool(name="castpool", bufs=15))
        opool = ctx.enter_context(tc.tile_pool(name="opool", bufs=2))

        def ps_tile(name):
            return psum.tile([P, TQ], F32, name=name, tag="ps")

        def row_mm(out_tile, h, lhsT, rhs):
            # ones-matmul partition reduction into 32-aligned row 32*h.
            # Each row-MM is its own complete accumulation group (rows are
            # disjoint; a shared group would accumulate onto stale bits).
            tp = (0, 32 * h) if h == 3 else None
            nc.tensor.matmul(out_tile[32 * h:32 * h + 1, :], lhsT, rhs,
                             start=True, stop=True, tile_position=tp)

        # ---- warm-up primer: dep-free dense matmuls so the PE's HAM clock
        # gate is released before the first real matmuls arrive ----
        prime_sb = consts.tile([P, TQ], BF16, name="prime_sb")
        nc.vector.memset(prime_sb[:], 0.001)
        prime_w = consts.tile([P, P], BF16, name="prime_w")
        nc.vector.memset(prime_w[:], 0.001)
        _burst_n = [0]

        def warm_burst(n):
            _burst_n[0] += 1
            bp = ps_tile(f"warm_ps{_burst_n[0]}")
            for i in range(n):
                nc.tensor.matmul(bp[:], prime_w[:], prime_sb[:],
                                 start=(i == 0), stop=(i == n - 1))

        warm_burst(16)

        # ---- constants ----
        cos2_sb = consts.tile([P, T], BF16, name="cos2_sb")
        nc.scalar.dma_start(out=cos2_sb[:], in_=cos2[:])
        sin2n_sb = consts.tile([P, T], BF16, name="sin2n_sb")
        nc.scalar.dma_start(out=sin2n_sb[:], in_=sin2n[:])
        tri_sb = consts.tile([P, P], BF16, name="tri_sb")
        nc.scalar.dma_start(out=tri_sb[:], in_=tri[:])
        ones_red = consts.tile([P, 1], BF16, name="ones_red")
        nc.vector.memset(ones_red[:], 1.0)
        zeros128 = consts.tile([P, 1], F32, name="zeros128")
        nc.vector.memset(zeros128[:], 0.0)
        eps_k128 = consts.tile([P, 1], F32, name="eps_k128")
        nc.vector.memset(eps_k128[:], EPS)
        eps_q128 = consts.tile([P, 1], F32, name="eps_q128")
        nc.vector.memset(eps_q128[:], float(D) * EPS)
        sels = []
        for j in range(4):
            s = consts.tile([P, P], BF16, name=f"sel{j}")
            nc.vector.memset(s[:], 0.0)
            nc.vector.memset(s[32 * j:32 * j + 1, :], 1.0)
            sels.append(s)

        # ---- up-projection + output weights (resident; single big DMAs) ----
        wu = ctx.enter_context(tc.tile_pool(name="wu", bufs=1))
        wkb_sb = wu.tile([P, NC_C, HG * D], BF16, name="wkb_sb")
        nc.scalar.dma_start(out=wkb_sb[:], in_=wkb[:])
        wkpe_sb = wu.tile([P, NC_P, HG * D], BF16, name="wkpe_sb")
        nc.scalar.dma_start(out=wkpe_sb[:], in_=wkpe[:])
        wv_sb = wu.tile([P, NC_C, HG * D], BF16, name="wv_sb")
        nc.scalar.dma_start(out=wv_sb[:], in_=wvb[:])
        wqb_sb = wu.tile([P, NLB, HG * D], BF16, name="wqb_sb")
        wo_sb = wu.tile([P, HG, C], BF16, name="wo_sb")

        # ---- persistent phase products ----
        yTn_sb = persist.tile([P, HG, T], BF16, name="yTn_sb")
        kTn_sb = persist.tile([P, HG, T], BF16, name="kTn_sb")
        qTn_sb = persist.tile([P, HG, T], BF16, name="qTn_sb")
        v_sb = persist.tile([P, NKT, HG * D], BF16, name="v_sb")

        # ---- phase L: latent projections + 3 AllGathers ----
        cc_out = {}

        def latent_pass(wname, wh, ngrp, lwp, lsp):
            """ngrp output col-groups of 128; contraction over 16 x-chunks.
            Copies PSUM->SBUF, DMAs to ccin (gpsimd queue), AllGathers."""
            pss = [ps_tile(f"lat_ps_{wname}{g}") for g in range(ngrp)]
            for cc in range(NCC):
                wt = lwp.tile([P, ngrp * P], BF16, name=f"wt_{wname}{cc}", tag="wt")
                nc.sync.dma_start(out=wt[:], in_=wh[:, cc, :])
                for g in range(ngrp):
                    nc.tensor.matmul(pss[g][:], wt[:, g * P:(g + 1) * P],
                                     xsb[:, cc, :], start=(cc == 0),
                                     stop=(cc == NCC - 1))
            lat = lsp.tile([P, ngrp, TQ], BF16, name=f"lat_{wname}", tag="lat")
            for g in range(ngrp):
                nc.scalar.copy(out=lat[:, g, :], in_=pss[g][:])
            ccin = dram.tile([P, ngrp * TQ], BF16, name=f"cc_in_{wname}",
                             tag=f"cc_in_{wname}")
            ccout = dram.tile([4 * P, ngrp * TQ], BF16, name=f"cc_out_{wname}",
                              tag=f"cc_out_{wname}")
            cc_out[wname] = ccout
            nc.scalar.dma_start(out=ccin[:], in_=lat[:])
            nc.gpsimd.collective_compute(
                "AllGather", mybir.AluOpType.bypass, replica_groups=RG,
                ins=[ccin.opt()], outs=[ccout.opt()])

        with tc.tile_pool(name="latw", bufs=4) as lwp, \
             tc.tile_pool(name="latstage", bufs=2) as lsp, \
             tc.tile_pool(name="xpool", bufs=1) as xpool:
            # x quarter: split load (first 4 chunks, then 12) so MMs start early
            xsb = xpool.tile([P, NCC, TQ], BF16, name="xsb")
            for c0 in range(0, NCC, 4):
                nc.sync.dma_start(out=xsb[:, c0:c0 + 4, :], in_=xh[:, c0:c0 + 4, :])
            latent_pass("kvc", wkvc, NC_C, lwp, lsp)
            nc.scalar.dma_start(out=wqb_sb[:], in_=wqb[:])
            latent_pass("kvp", wkvp, NC_P, lwp, lsp)
            nc.scalar.dma_start(out=wo_sb[:], in_=wo[:])
            latent_pass("q", wqa, NLB, lwp, lsp)

        def load_lat(pool, wname, name, tt, ngrp):
            t = pool.tile([P, ngrp, TQ], BF16, name=name, tag=pool.name)
            nc.sync.dma_start(out=t[:],
                              in_=cc_out[wname][tt * P:(tt + 1) * P, :])
            return t

        # ---- K/V machinery ----
        def kc_pass(tt, kvc_t, kcp):
            # k-content for 4 heads + V for 4 token-128s (content gather only)
            kc_sb = kcp.tile([P, HG, TQ], BF16, name=f"kc_sb{tt}", tag="kc")
            for h in range(HG):
                kc_ps = ps_tile(f"kc_ps_{h}_{tt}")
                for j in range(NC_C):
                    nc.tensor.matmul(kc_ps[:], wkb_sb[:, j, h * D:(h + 1) * D],
                                     kvc_t[:, j, :], start=(j == 0),
                                     stop=(j == NC_C - 1))
                nc.scalar.copy(out=kc_sb[:, h, :], in_=kc_ps[:])
                v_ps = ps_tile(f"v_ps_{tt}_{h}")
                for j in range(NC_C):
                    nc.tensor.matmul(v_ps[:], kvc_t[:, j, h * P:(h + 1) * P],
                                     wv_sb[:, j, :], start=(j == 0),
                                     stop=(j == NC_C - 1))
                nc.scalar.copy(out=v_sb[:, tt * 4 + h, :], in_=v_ps[:])
            return kc_sb

        def kpe_mms(tt, kvp_t, kc_sb):
            # k_pe up-proj + rope + assemble k_un + squares (chains on DVE)
            sqs = []
            hd = D // 2
            for h in range(HG):
                kpe_ps = ps_tile(f"kpe_ps_{h}_{tt}")
                for j in range(NC_P):
                    nc.tensor.matmul(kpe_ps[:], wkpe_sb[:, j, h * D:(h + 1) * D],
                                     kvp_t[:, j, :], start=(j == 0), stop=(j == 1))
                # kswap = halves of kpe swapped (PSUM reads may cross partitions)
                kswap = ropep.tile([P, TQ], BF16, name=f"kswap_{h}_{tt}", tag="rope")
                nc.scalar.copy(out=kswap[0:hd, :], in_=kpe_ps[hd:D, :])
                nc.scalar.copy(out=kswap[hd:D, :], in_=kpe_ps[0:hd, :])
                t1 = ropep.tile([P, TQ], BF16, name=f"t1_{h}_{tt}", tag="rope")
                nc.vector.tensor_mul(t1[:], kpe_ps[:], cos2_sb[:, tt * TQ:(tt + 1) * TQ])
                t2 = ropep.tile([P, TQ], BF16, name=f"t2_{h}_{tt}", tag="rope")
                nc.vector.tensor_mul(t2[:], kswap[:], sin2n_sb[:, tt * TQ:(tt + 1) * TQ])
                nc.vector.tensor_add(t1[:], t1[:], t2[:])
                k_un = kunp.tile([P, TQ], BF16, name=f"k_un_{h}_{tt}", tag="kun")
                nc.vector.tensor_add(k_un[:], t1[:], kc_sb[:, h, :])
                sq = tmpsq.tile([P, TQ], BF16, name=f"ksq_{h}_{tt}", tag="sq")
                nc.vector.tensor_mul(sq[:], k_un[:], k_un[:])
                sqs.append((k_un, sq))
            return sqs

        def k_tail(tt, sqs):
            # emitted behind filler MMs so the DVE chains have drained
            ss_k = ps_tile(f"ss_k_{tt}")
            nc.vector.memset(ss_k[:], 1.0)
            for h in range(HG):
                row_mm(ss_k, h, ones_red[:], sqs[h][1][:])
            rbf = rsqrt_tail(ss_k, 1.0 / D, eps_k128, f"k_{tt}")
            for h in range(HG):
                bc = ps_tile(f"kbc_{h}_{tt}")
                nc.tensor.matmul(bc[:], sels[h][:], rbf[:], start=True, stop=True)
                nc.vector.tensor_mul(kTn_sb[:, h, tt * TQ:(tt + 1) * TQ],
                                     sqs[h][0][:], bc[:])

        # ---- Q sections ----
        def q_mms(tt, qlsb_t):
            qps = []
            for h in range(HG):
                q_ps = ps_tile(f"q_ps_{h}_{tt}")
                qps.append(q_ps)
                for j in range(NLB // 2):
                    nc.tensor.matmul(q_ps[:], wqb_sb[:, j, h * D:(h + 1) * D],
                                     qlsb_t[:, j, :], start=(j == 0), stop=False)
            qcs = []
            sqs = []
            for h in range(HG):
                q_ps = qps[h]
                for j in range(NLB // 2, NLB):
                    nc.tensor.matmul(q_ps[:], wqb_sb[:, j, h * D:(h + 1) * D],
                                     qlsb_t[:, j, :], start=False, stop=(j == NLB - 1))
                qc = castpool.tile([P, TQ], BF16, name=f"qc_{h}_{tt}", tag="cast")
                nc.scalar.copy(out=qc[:], in_=q_ps[:])
                qcs.append(qc)
                sq = tmpsq.tile([P, TQ], BF16, name=f"qsq_{h}_{tt}", tag="sq")
                nc.scalar.activation(sq[:], q_ps[:], AF.Square, bias=zeros128[:], scale=1.0)
                sqs.append(sq)
            return qcs, sqs

        def q_tail(tt, qcs, sqs):
            ss_q = ps_tile(f"ss_q_{tt}")
            nc.vector.memset(ss_q[:], 1.0)
            for h in range(HG):
                row_mm(ss_q, h, ones_red[:], sqs[h][:])
            # rbf = 1/sqrt(ss + D*eps): folds the 1/sqrt(D) attention scale
            rbf = rsqrt_tail(ss_q, 1.0, eps_q128, f"q_{tt}")
            for h in range(HG):
                bc = ps_tile(f"qbc_{h}_{tt}")
                nc.tensor.matmul(bc[:], sels[h][:], rbf[:], start=True, stop=True)
                nc.vector.tensor_mul(qTn_sb[:, h, tt * TQ:(tt + 1) * TQ],
                                     qcs[h][:], bc[:])

        # ---- attention ----
        def rsqrt_tail(ss, scale, bias, which):
            # bf16 1/sqrt(ss*scale + bias) via ScalarE Sqrt -> DVE approx-recip
            sroot = rinvp.tile([P, TQ], F32, name=f"sroot_{which}", tag="rinv")
            nc.scalar.activation(sroot[:], ss[:], AF.Sqrt, bias=bias[:], scale=scale)
            rinv = rinvp.tile([P, TQ], F32, name=f"rinv_{which}", tag="rinv")
            nc.vector.reciprocal_approx_fast(out=rinv[:], in_=sroot[:])
            rbf = rbfp.tile([P, TQ], BF16, name=f"rbf_{which}", tag="rbf")
            nc.scalar.copy(out=rbf[:], in_=rinv[:])
            return rbf

        def a_tail(qb, den4, ycs):
            rinv = rinvp.tile([P, TQ], F32, name=f"rden_{qb}", tag="rinv")
            nc.vector.reciprocal_approx_fast(out=rinv[:], in_=den4[:])
            rbf = rbfp.tile([P, TQ], BF16, name=f"rdenb_{qb}", tag="rbf")
            nc.scalar.copy(out=rbf[:], in_=rinv[:])
            for h in range(HG):
                bc = ps_tile(f"abc_{h}_{qb}")
                nc.tensor.matmul(bc[:], sels[h][:], rbf[:], start=True, stop=True)
                nc.vector.tensor_mul(yTn_sb[:, h, qb * TQ:(qb + 1) * TQ],
                                     ycs[h][:], bc[:])

        pending_a = []

        def a_block(qb):
            # memset to 1.0 (not 0): unused rows go through reciprocal
            den4 = ps_tile(f"den4_{qb}")
            nc.vector.memset(den4[:], 1.0)
            ycs = []
            nkt = 4 * (qb + 1)
            for h in range(HG):
                yt_ps = ps_tile(f"yt_ps_{h}_{qb}")
                acc = accpool.tile([P, TQ], BF16, name=f"acc_{h}_{qb}", tag="acc")

                def emit_sc(kt):
                    # columns < P*jrel of the diagonal tiles are fully masked:
                    # trim them from the score MM, exp, and PV/acc
                    jrel = kt - 4 * qb
                    c0 = P * jrel if jrel > 0 else 0
                    sc_ps = ps_tile(f"sc_ps_{h}_{qb}_{kt}")
                    nc.tensor.matmul(sc_ps[:, c0:], kTn_sb[:, h, kt * P:(kt + 1) * P],
                                     qTn_sb[:, h, qb * TQ + c0:(qb + 1) * TQ],
                                     start=True, stop=True)
                    ex = expool.tile([P, TQ], BF16, name=f"ex_{h}_{qb}_{kt}", tag="ex")
                    nc.scalar.activation(ex[:, c0:], sc_ps[:, c0:], AF.Exp,
                                         bias=zeros128[:], scale=1.0)
                    if jrel >= 0:
                        if jrel > 0:
                            nc.vector.memset(ex[:, 0:c0], 0.0)
                        nc.vector.tensor_mul(ex[:, c0:c0 + P], ex[:, c0:c0 + P],
                                             tri_sb[:])
                    return (ex, c0)

                def emit_pv(kt, ex, c0):
                    if kt == 0:
                        nc.vector.tensor_copy(out=acc[:], in_=ex[:])
                    else:
                        nc.vector.tensor_add(acc[:, c0:], acc[:, c0:], ex[:, c0:])
                    nc.tensor.matmul(yt_ps[:, c0:], v_sb[:, kt, h * D:(h + 1) * D],
                                     ex[:, c0:], start=(kt == 0),
                                     stop=(kt == nkt - 1))

                # 3-deep lookahead: scores for kt+1..kt+3 are issued before
                # pv(kt) so each pv's exp is ready when the in-order PE
                # reaches it
                exs = {}
                for k0 in range(min(3, nkt)):
                    exs[k0] = emit_sc(k0)
                for kt in range(nkt):
                    if kt + 3 < nkt:
                        exs[kt + 3] = emit_sc(kt + 3)
                    emit_pv(kt, *exs.pop(kt))
                row_mm(den4, h, ones_red[:], acc[:])
                yc = castpool.tile([P, TQ], BF16, name=f"yc_{h}_{qb}", tag="cast")
                nc.scalar.copy(out=yc[:], in_=yt_ps[:])
                ycs.append(yc)
                if pending_a and h == 1:
                    a_tail(*pending_a.pop(0))
            pending_a.append((qb, den4, ycs))

        # ---- output projection, one token-tile ----
        def o_pass(tt):
            for ct in range(C // P):
                o_ps = ps_tile(f"o_ps_{ct}_{tt}")
                for h in range(HG):
                    nc.tensor.matmul(o_ps[:], wo_sb[:, h, ct * P:(ct + 1) * P],
                                     yTn_sb[:, h, tt * TQ:(tt + 1) * TQ],
                                     start=(h == 0), stop=(h == HG - 1))
                o_sb = opool.tile([P, TQ], BF16, name=f"o_sb_{ct}_{tt}", tag="o_sb")
                nc.vector.tensor_copy(out=o_sb[:], in_=o_ps[:])
                nc.sync.dma_start(out=outT[ct * P:(ct + 1) * P, tt * TQ:(tt + 1) * TQ],
                                  in_=o_sb[:])

        # ---- emission: K/V content pass, pe pass, then Q woven with A/O ----
        with tc.tile_pool(name="kvcpool", bufs=2) as kvcpool, \
             tc.tile_pool(name="kvppool", bufs=2) as kvppool, \
             tc.tile_pool(name="kcsbp", bufs=2) as kcsbp, \
             tc.tile_pool(name="qlpool", bufs=2) as qlpool:
            warm_burst(24)
            # weave: every norm tail is emitted behind a filler MM pass so
            # the DVE/ScalarE dependency chains drain off the PE critical path
            kc_sbs = {}
            k_sqs = {}
            for tt in range(NTT):
                kvc_t = load_lat(kvcpool, "kvc", f"kvcsb{tt}", tt, NC_C)
                kc_sbs[tt] = kc_pass(tt, kvc_t, kcsbp)
                if tt >= 1:
                    kvp_t = load_lat(kvppool, "kvp", f"kvpsb{tt-1}", tt - 1, NC_P)
                    k_sqs[tt - 1] = kpe_mms(tt - 1, kvp_t, kc_sbs.pop(tt - 1))
                if tt >= 2:
                    k_tail(tt - 2, k_sqs.pop(tt - 2))
            kvp_t = load_lat(kvppool, "kvp", f"kvpsb{NTT-1}", NTT - 1, NC_P)
            k_sqs[NTT - 1] = kpe_mms(NTT - 1, kvp_t, kc_sbs.pop(NTT - 1))
            k_tail(NTT - 2, k_sqs.pop(NTT - 2))
            warm_burst(12)
            k_tail(NTT - 1, k_sqs.pop(NTT - 1))

            q_state = {}
            for tt in range(NTT):
                qlsb_t = load_lat(qlpool, "q", f"qlsb{tt}", tt, NLB)
                q_state[tt] = q_mms(tt, qlsb_t)
                if tt >= 1:
                    q_tail(tt - 1, *q_state.pop(tt - 1))
                    a_block(tt - 1)
                    if tt >= 2:
                        o_pass(tt - 2)
            q_tail(NTT - 1, *q_state.pop(NTT - 1))
            a_block(NTT - 1)
            o_pass(NTT - 2)
            a_tail(*pending_a.pop(0))
            o_pass(NTT - 1)

    nc.compile()
    return nc


def _get_nc():
    if "nc" not in _NC_CACHE:
        _NC_CACHE["nc"] = build_nc()
    return _NC_CACHE["nc"]


def _prepare_in_maps(x, cos, sin, wq_a, wq_b, wkv_a, wk_b, wkpe_b, wv_b, wo):
    def bf(a):
        return np.ascontiguousarray(a).astype(NPBF16)

    cosT = np.asarray(cos, np.float32)[0, :, 0, :].T   # (64, T)
    sinT = np.asarray(sin, np.float32)[0, :, 0, :].T
    cos2 = bf(np.concatenate([cosT, cosT], axis=0))    # (128, T)
    sin2n = bf(np.concatenate([sinT, -sinT], axis=0))
    tri = (np.arange(P)[:, None] <= np.arange(P)[None, :]).astype(NPBF16)

    x = np.asarray(x, np.float32)
    wkv_a = np.asarray(wkv_a, np.float32)
    wq_a = np.asarray(wq_a, np.float32)
    # (j p) m -> p j m relayouts (partition-contiguous DRAM)
    wkvc_h = bf(wkv_a.reshape(NCC, P, LORA)[:, :, KV_PE:].transpose(1, 0, 2))
    wkvp_h = bf(wkv_a.reshape(NCC, P, LORA)[:, :, :KV_PE].transpose(1, 0, 2))
    wqa_h = bf(wq_a.reshape(NCC, P, LORA).transpose(1, 0, 2))

    def up_relay(w, k):
        return bf(np.asarray(w, np.float32).reshape(k, P, -1).transpose(1, 0, 2))

    in_maps = []
    for c in range(8):
        b, r = c // 4, c % 4
        xq = x[b, r * TQ:(r + 1) * TQ, :]                  # (512, 2048)
        xh = bf(xq.reshape(TQ, NCC, P).transpose(2, 1, 0))  # (128, 16, 512)
        hgs = slice(r * HG * D, (r + 1) * HG * D)
        wo_c = np.asarray(wo, np.float32)[hgs, :]          # (512, 2048)
        wo_h = bf(wo_c.reshape(HG, P, C).transpose(1, 0, 2))
        in_maps.append({
            "xh": xh,
            "wkvc": wkvc_h,
            "wkvp": wkvp_h,
            "wqa": wqa_h,
            "wkb": up_relay(np.asarray(wk_b, np.float32)[:, hgs], NC_C),
            "wkpe": up_relay(np.asarray(wkpe_b, np.float32)[:, hgs], NC_P),
            "wvb": up_relay(np.asarray(wv_b, np.float32)[:, hgs], NC_C),
            "wqb": up_relay(np.asarray(wq_b, np.float32)[:, hgs], NLB),
            "wo": wo_h,
            "cos2": cos2,
            "sin2n": sin2n,
            "tri": tri,
        })
    return in_maps


def _assemble(results):
    out = np.empty((B, T, C), np.float32)
    for b in range(B):
        acc = results[4 * b]["outT"].astype(np.float32)
        for r in range(1, 4):
            acc = acc + results[4 * b + r]["outT"].astype(np.float32)
        out[b] = acc.T
    return out


def _run(inputs, trace=False):
    nc = _get_nc()
    in_maps = _prepare_in_maps(**inputs)
    res = run_bass_kernel_spmd(nc, in_maps, core_ids=list(range(8)), trace=trace)
    return _assemble(res.results), res


def kernel(**inputs):
    out, _ = _run(inputs)
    return out


# revision 14
# speedup vs baseline: 1.0445x; 1.0004x over previous
"""Trainium2 Bass kernel for MultiHeadLatentAttention (B=2, T=2048, C=2048, 16 heads).

Sharding over 8 NeuronCores: core c = (batch b = c//4, r = c%4).
 - Latent projections (x@wq_a, x@wkv_a) computed token-sharded (quarter r),
   latent-dim on partitions, then AllGather-ed within each 4-core batch
   group as THREE gathers ordered by first need: kv-content (0.75MB),
   kv-pe (0.25MB), q (1MB).  The content half of the kv latent is computed
   first so its gather starts ~25us earlier than a combined gather would.
 - Each core then handles head-group r (4 of 16 heads) for the full
   sequence: up-projections, RoPE+RMSNorm, block-causal attention, and a
   row-shard of the output projection.  Host sums the 4 partials per batch.

All matmuls bf16 with fp32 PSUM accumulation.  RMS/softmax reciprocal
tails run on ScalarE (Rsqrt/Reciprocal activations, output bf16 direct);
per-row broadcasts are selector-matmuls.  All DRAM inputs are host-relaid
partition-contiguous so every load is one big-descriptor DMA.  Score/exp/
PV work on causal-diagonal tiles is column-trimmed.  The output projection
is woven per token-tile into the attention phase.
"""

from contextlib import ExitStack

import numpy as np
import ml_dtypes

import concourse.bass as bass
import concourse.tile as tile
import concourse.mybir as mybir
from concourse import bacc
from concourse.bass_utils import run_bass_kernel_spmd

BF16 = mybir.dt.bfloat16
F32 = mybir.dt.float32
NPBF16 = ml_dtypes.bfloat16
AF = mybir.ActivationFunctionType

P = 128
B, T, C = 2, 2048, 2048
H, D = 16, 128
LORA = 1024
KV_PE = 256           # latent chunks 0-1
CONTENT = 768         # latent chunks 2-7
NC_C = CONTENT // P   # 6
NC_P = KV_PE // P     # 2
EPS = 1.1920929e-07
HG = 4                # heads per core
TQ = 512              # tokens per quarter / query block
NLB = LORA // P       # 8 latent row-blocks
NCC = C // P          # 16 contraction chunks of x
NTT = T // TQ         # 4 token 512-tiles
NKT = T // P          # 16 key tiles of 128
RG = [[0, 1, 2, 3], [4, 5, 6, 7]]

_NC_CACHE = {}


def build_nc():
    nc = bacc.Bacc("TRN2", target_bir_lowering=False, debug=False, num_devices=8)

    # all host-relaid partition-contiguous (see _prepare_in_maps)
    xh = nc.dram_tensor("xh", [P, NCC, TQ], BF16, kind="ExternalInput")
    wkva = nc.dram_tensor("wkva", [P, NCC, LORA], BF16, kind="ExternalInput")
    wqa = nc.dram_tensor("wqa", [P, NCC, LORA], BF16, kind="ExternalInput")
    wkb = nc.dram_tensor("wkb", [P, NC_C, HG * D], BF16, kind="ExternalInput")
    wkpe = nc.dram_tensor("wkpe", [P, NC_P, HG * D], BF16, kind="ExternalInput")
    wvb = nc.dram_tensor("wvb", [P, NC_C, HG * D], BF16, kind="ExternalInput")
    wqb = nc.dram_tensor("wqb", [P, NLB, HG * D], BF16, kind="ExternalInput")
    wo = nc.dram_tensor("wo", [P, HG, C], BF16, kind="ExternalInput")
    cos2 = nc.dram_tensor("cos2", [P, T], BF16, kind="ExternalInput")
    sin2n = nc.dram_tensor("sin2n", [P, T], BF16, kind="ExternalInput")
    tri = nc.dram_tensor("tri", [P, P], BF16, kind="ExternalInput")
    outT = nc.dram_tensor("outT", [C, T], BF16, kind="ExternalOutput")

    with tile.TileContext(nc) as tc, ExitStack() as ctx:
        dram = ctx.enter_context(tc.tile_pool(name="dram", bufs=1, space="DRAM"))
        psum = ctx.enter_context(tc.tile_pool(name="psum", bufs=8, space="PSUM"))
        consts = ctx.enter_context(tc.tile_pool(name="consts", bufs=1))
        persist = ctx.enter_context(tc.tile_pool(name="persist", bufs=1))
        tmpsq = ctx.enter_context(tc.tile_pool(name="tmpsq", bufs=8))
        ropep = ctx.enter_context(tc.tile_pool(name="ropep", bufs=4))
        kunp = ctx.enter_context(tc.tile_pool(name="kunp", bufs=8))
        rbfp = ctx.enter_context(tc.tile_pool(name="rbfp", bufs=2))
        rinvp = ctx.enter_context(tc.tile_pool(name="rinvp", bufs=2))
        expool = ctx.enter_context(tc.tile_pool(name="expool", bufs=4))
        accpool = ctx.enter_context(tc.tile_pool(name="accpool", bufs=4))
        castpool = ctx.enter_context(tc.tile_pool(name="castpool", bufs=15))
        opool = ctx.enter_context(tc.tile_pool(name="opool", bufs=2))

        def ps_tile(name):
            return psum.tile([P, TQ], F32, name=name, tag="ps")

        def row_mm(out_tile, h, lhsT, rhs):
            # ones-matmul partition reduction into 32-aligned row 32*h.
            # Each row-MM is its own complete accumulation group (rows are
            # disjoint; a shared group would accumulate onto stale bits).
            tp = (0, 32 * h) if h == 3 else None
            nc.tensor.matmul(out_tile[32 * h:32 * h + 1, :], lhsT, rhs,
                             start=True, stop=True, tile_position=tp)

        # ---- warm-up primer: dep-free dense matmuls so the PE's HAM clock
        # gate is released before the first real matmuls arrive ----
        prime_sb = consts.tile([P, TQ], BF16, name="prime_sb")
        nc.vector.memset(prime_sb[:], 0.001)
        prime_w = consts.tile([P, P], BF16, name="prime_w")
        nc.vector.memset(prime_w[:], 0.001)
        _burst_n = [0]

        def warm_burst(n):
            _burst_n[0] += 1
            bp = ps_tile(f"warm_ps{_burst_n[0]}")
            for i in range(n):
                nc.tensor.matmul(bp[:], prime_w[:], prime_sb[:],
                                 start=(i == 0), stop=(i == n - 1))

        warm_burst(16)

        # ---- constants ----
        cos2_sb = consts.tile([P, T], BF16, name="cos2_sb")
        nc.gpsimd.dma_start(out=cos2_sb[:], in_=cos2[:])
        sin2n_sb = consts.tile([P, T], BF16, name="sin2n_sb")
        nc.gpsimd.dma_start(out=sin2n_sb[:], in_=sin2n[:])
        tri_sb = consts.tile([P, P], BF16, name="tri_sb")
        nc.gpsimd.dma_start(out=tri_sb[:], in_=tri[:])
        ones_red = consts.tile([P, 1], BF16, name="ones_red")
        nc.vector.memset(ones_red[:], 1.0)
        zeros128 = consts.tile([P, 1], F32, name="zeros128")
        nc.vector.memset(zeros128[:], 0.0)
        eps_k128 = consts.tile([P, 1], F32, name="eps_k128")
        nc.vector.memset(eps_k128[:], EPS)
        eps_q128 = consts.tile([P, 1], F32, name="eps_q128")
        nc.vector.memset(eps_q128[:], float(D) * EPS)
        sels = []
        for j in range(4):
            s = consts.tile([P, P], BF16, name=f"sel{j}")
            nc.vector.memset(s[:], 0.0)
            nc.vector.memset(s[32 * j:32 * j + 1, :], 1.0)
            sels.append(s)

        # ---- up-projection + output weights (resident; single big DMAs) ----
        wu = ctx.enter_context(tc.tile_pool(name="wu", bufs=1))
        wkb_sb = wu.tile([P, NC_C, HG * D], BF16, name="wkb_sb")
        nc.gpsimd.dma_start(out=wkb_sb[:], in_=wkb[:])
        wkpe_sb = wu.tile([P, NC_P, HG * D], BF16, name="wkpe_sb")
        nc.gpsimd.dma_start(out=wkpe_sb[:], in_=wkpe[:])
        wv_sb = wu.tile([P, NC_C, HG * D], BF16, name="wv_sb")
        nc.gpsimd.dma_start(out=wv_sb[:], in_=wvb[:])
        wqb_sb = wu.tile([P, NLB, HG * D], BF16, name="wqb_sb")
        nc.gpsimd.dma_start(out=wqb_sb[:], in_=wqb[:])
        wo_sb = wu.tile([P, HG, C], BF16, name="wo_sb")
        nc.gpsimd.dma_start(out=wo_sb[:], in_=wo[:])

        # ---- persistent phase products ----
        yTn_sb = persist.tile([P, HG, T], BF16, name="yTn_sb")
        kTn_sb = persist.tile([P, HG, T], BF16, name="kTn_sb")
        qTn_sb = persist.tile([P, HG, T], BF16, name="qTn_sb")
        v_sb = persist.tile([P, NKT, HG * D], BF16, name="v_sb")

        # ---- phase L: latent projections + 3 AllGathers ----
        cc_out = {}

        def latent_pass(wname, wh, c0, c1, lwp, lsp):
            """col-groups [c0:c1] of 128; contraction over 16 x-chunks.
            Copies PSUM->SBUF, DMAs to ccin (scalar queue), AllGathers."""
            ngrp = (c1 - c0) // P
            pss = [ps_tile(f"lat_ps_{wname}{g}") for g in range(ngrp)]
            for cc in range(NCC):
                wt = lwp.tile([P, ngrp * P], BF16, name=f"wt_{wname}{cc}", tag="wt")
                nc.sync.dma_start(out=wt[:], in_=wh[:, cc, c0:c1])
                for g in range(ngrp):
                    nc.tensor.matmul(pss[g][:], wt[:, g * P:(g + 1) * P],
                                     xsb[:, cc, :], start=(cc == 0),
                                     stop=(cc == NCC - 1))
            lat = lsp.tile([P, ngrp, TQ], BF16, name=f"lat_{wname}", tag="lat")
            for g in range(ngrp):
                nc.scalar.copy(out=lat[:, g, :], in_=pss[g][:])
            ccin = dram.tile([P, ngrp * TQ], BF16, name=f"cc_in_{wname}",
                             tag=f"cc_in_{wname}")
            ccout = dram.tile([4 * P, ngrp * TQ], BF16, name=f"cc_out_{wname}",
                              tag=f"cc_out_{wname}")
            cc_out[wname] = ccout
            nc.scalar.dma_start(out=ccin[:], in_=lat[:])
            nc.gpsimd.collective_compute(
                "AllGather", mybir.AluOpType.bypass, replica_groups=RG,
                ins=[ccin.opt()], outs=[ccout.opt()])

        with tc.tile_pool(name="latw", bufs=4) as lwp, \
             tc.tile_pool(name="latstage", bufs=2) as lsp, \
             tc.tile_pool(name="xpool", bufs=1) as xpool:
            # x quarter: split load (first 4 chunks, then 12) so MMs start early
            xsb = xpool.tile([P, NCC, TQ], BF16, name="xsb")
            for c0 in range(0, NCC, 4):
                nc.sync.dma_start(out=xsb[:, c0:c0 + 4, :], in_=xh[:, c0:c0 + 4, :])
            latent_pass("c1", wkva, 0, 3 * P, lwp, lsp)
            latent_pass("c2", wkva, 3 * P, LORA, lwp, lsp)
            latent_pass("q", wqa, 0, LORA, lwp, lsp)

        def load_lat(pool, wname, name, tt, ngrp):
            t = pool.tile([P, ngrp, TQ], BF16, name=name, tag=pool.name)
            nc.sync.dma_start(out=t[:],
                              in_=cc_out[wname][tt * P:(tt + 1) * P, :])
            return t

        # ---- K/V machinery (content chunks 0-2 in kv1, 3-5 + pe in kv2) ----
        def kc_a(tt, kv1_t):
            kcps = []
            for h in range(HG):
                kc_ps = ps_tile(f"kc_ps_{h}_{tt}")
                for j in range(3):
                    nc.tensor.matmul(kc_ps[:], wkb_sb[:, j, h * D:(h + 1) * D],
                                     kv1_t[:, j, :], start=(j == 0), stop=False)
                kcps.append(kc_ps)
            return kcps

        def kc_b(tt, kv2_t, kcps, kcp):
            kc_sb = kcp.tile([P, HG, TQ], BF16, name=f"kc_sb{tt}", tag="kc")
            for h in range(HG):
                for j in range(3):
                    nc.tensor.matmul(kcps[h][:], wkb_sb[:, j + 3, h * D:(h + 1) * D],
                                     kv2_t[:, j, :], start=False, stop=(j == 2))
                nc.scalar.copy(out=kc_sb[:, h, :], in_=kcps[h][:])
            return kc_sb

        def v_pass(tt, kv1_t, kv2_t):
            for t4 in range(4):
                v_ps = ps_tile(f"v_ps_{tt}_{t4}")
                for j in range(3):
                    nc.tensor.matmul(v_ps[:], kv1_t[:, j, t4 * P:(t4 + 1) * P],
                                     wv_sb[:, j, :], start=(j == 0), stop=False)
                for j in range(3):
                    nc.tensor.matmul(v_ps[:], kv2_t[:, j, t4 * P:(t4 + 1) * P],
                                     wv_sb[:, j + 3, :], start=False, stop=(j == 2))
                nc.scalar.copy(out=v_sb[:, tt * 4 + t4, :], in_=v_ps[:])

        def kpe_mms(tt, kvp_t, kc_sb):
            # k_pe up-proj + rope + assemble k_un + squares (chains on DVE)
            sqs = []
            hd = D // 2
            for h in range(HG):
                kpe_ps = ps_tile(f"kpe_ps_{h}_{tt}")
                for j in range(NC_P):
                    nc.tensor.matmul(kpe_ps[:], wkpe_sb[:, j, h * D:(h + 1) * D],
                                     kvp_t[:, 3 + j, :], start=(j == 0), stop=(j == 1))
                # kswap = halves of kpe swapped (PSUM reads may cross partitions)
                kswap = ropep.tile([P, TQ], BF16, name=f"kswap_{h}_{tt}", tag="rope")
                nc.scalar.copy(out=kswap[0:hd, :], in_=kpe_ps[hd:D, :])
                nc.scalar.copy(out=kswap[hd:D, :], in_=kpe_ps[0:hd, :])
                t1 = ropep.tile([P, TQ], BF16, name=f"t1_{h}_{tt}", tag="rope")
                nc.vector.tensor_mul(t1[:], kpe_ps[:], cos2_sb[:, tt * TQ:(tt + 1) * TQ])
                t2 = ropep.tile([P, TQ], BF16, name=f"t2_{h}_{tt}", tag="rope")
                nc.vector.tensor_mul(t2[:], kswap[:], sin2n_sb[:, tt * TQ:(tt + 1) * TQ])
                nc.vector.tensor_add(t1[:], t1[:], t2[:])
                k_un = kunp.tile([P, TQ], BF16, name=f"k_un_{h}_{tt}", tag="kun")
                nc.vector.tensor_add(k_un[:], t1[:], kc_sb[:, h, :])
                sq = tmpsq.tile([P, TQ], BF16, name=f"ksq_{h}_{tt}", tag="sq")
                nc.vector.tensor_mul(sq[:], k_un[:], k_un[:])
                sqs.append((k_un, sq))
            return sqs

        def k_tail(tt, sqs):
            # emitted behind filler MMs so the DVE chains have drained
            ss_k = ps_tile(f"ss_k_{tt}")
            nc.vector.memset(ss_k[:], 1.0)
            for h in range(HG):
                row_mm(ss_k, h, ones_red[:], sqs[h][1][:])
            rbf = rsqrt_tail(ss_k, 1.0 / D, eps_k128, f"k_{tt}")
            for h in range(HG):
                bc = ps_tile(f"kbc_{h}_{tt}")
                nc.tensor.matmul(bc[:], sels[h][:], rbf[:], start=True, stop=True)
                nc.vector.tensor_mul(kTn_sb[:, h, tt * TQ:(tt + 1) * TQ],
                                     sqs[h][0][:], bc[:])

        # ---- Q sections ----
        def q_mms(tt, qlsb_t):
            qps = []
            for h in range(HG):
                q_ps = ps_tile(f"q_ps_{h}_{tt}")
                qps.append(q_ps)
                for j in range(NLB // 2):
                    nc.tensor.matmul(q_ps[:], wqb_sb[:, j, h * D:(h + 1) * D],
                                     qlsb_t[:, j, :], start=(j == 0), stop=False)
            qcs = []
            sqs = []
            for h in range(HG):
                q_ps = qps[h]
                for j in range(NLB // 2, NLB):
                    nc.tensor.matmul(q_ps[:], wqb_sb[:, j, h * D:(h + 1) * D],
                                     qlsb_t[:, j, :], start=False, stop=(j == NLB - 1))
                qc = castpool.tile([P, TQ], BF16, name=f"qc_{h}_{tt}", tag="cast")
                nc.scalar.copy(out=qc[:], in_=q_ps[:])
                qcs.append(qc)
                sq = tmpsq.tile([P, TQ], BF16, name=f"qsq_{h}_{tt}", tag="sq")
                nc.scalar.activation(sq[:], q_ps[:], AF.Square, bias=zeros128[:], scale=1.0)
                sqs.append(sq)
            return qcs, sqs

        def q_tail(tt, qcs, sqs):
            ss_q = ps_tile(f"ss_q_{tt}")
            nc.vector.memset(ss_q[:], 1.0)
            for h in range(HG):
                row_mm(ss_q, h, ones_red[:], sqs[h][:])
            # rbf = 1/sqrt(ss + D*eps): folds the 1/sqrt(D) attention scale
            rbf = rsqrt_tail(ss_q, 1.0, eps_q128, f"q_{tt}")
            for h in range(HG):
                bc = ps_tile(f"qbc_{h}_{tt}")
                nc.tensor.matmul(bc[:], sels[h][:], rbf[:], start=True, stop=True)
                nc.vector.tensor_mul(qTn_sb[:, h, tt * TQ:(tt + 1) * TQ],
                                     qcs[h][:], bc[:])

        # ---- attention ----
        def rsqrt_tail(ss, scale, bias, which):
            # bf16 1/sqrt(ss*scale + bias) via ScalarE Sqrt -> DVE approx-recip
            sroot = rinvp.tile([P, TQ], F32, name=f"sroot_{which}", tag="rinv")
            nc.scalar.activation(sroot[:], ss[:], AF.Sqrt, bias=bias[:], scale=scale)
            rinv = rinvp.tile([P, TQ], F32, name=f"rinv_{which}", tag="rinv")
            nc.vector.reciprocal_approx_fast(out=rinv[:], in_=sroot[:])
            rbf = rbfp.tile([P, TQ], BF16, name=f"rbf_{which}", tag="rbf")
            nc.scalar.copy(out=rbf[:], in_=rinv[:])
            return rbf

        def a_tail(qb, den4, ycs):
            rinv = rinvp.tile([P, TQ], F32, name=f"rden_{qb}", tag="rinv")
            nc.vector.reciprocal_approx_fast(out=rinv[:], in_=den4[:])
            rbf = rbfp.tile([P, TQ], BF16, name=f"rdenb_{qb}", tag="rbf")
            nc.scalar.copy(out=rbf[:], in_=rinv[:])
            for h in range(HG):
                bc = ps_tile(f"abc_{h}_{qb}")
                nc.tensor.matmul(bc[:], sels[h][:], rbf[:], start=True, stop=True)
                nc.vector.tensor_mul(yTn_sb[:, h, qb * TQ:(qb + 1) * TQ],
                                     ycs[h][:], bc[:])

        pending_a = []

        def a_block(qb):
            # memset to 1.0 (not 0): unused rows go through reciprocal
            den4 = ps_tile(f"den4_{qb}")
            nc.vector.memset(den4[:], 1.0)
            ycs = []
            nkt = 4 * (qb + 1)
            for h in range(HG):
                yt_ps = ps_tile(f"yt_ps_{h}_{qb}")
                acc = accpool.tile([P, TQ], BF16, name=f"acc_{h}_{qb}", tag="acc")

                def emit_sc(kt):
                    # columns < P*jrel of the diagonal tiles are fully masked:
                    # trim them from the score MM, exp, and PV/acc
                    jrel = kt - 4 * qb
                    c0 = P * jrel if jrel > 0 else 0
                    sc_ps = ps_tile(f"sc_ps_{h}_{qb}_{kt}")
                    nc.tensor.matmul(sc_ps[:, c0:], kTn_sb[:, h, kt * P:(kt + 1) * P],
                                     qTn_sb[:, h, qb * TQ + c0:(qb + 1) * TQ],
                                     start=True, stop=True)
                    ex = expool.tile([P, TQ], BF16, name=f"ex_{h}_{qb}_{kt}", tag="ex")
                    nc.scalar.activation(ex[:, c0:], sc_ps[:, c0:], AF.Exp,
                                         bias=zeros128[:], scale=1.0)
                    if jrel >= 0:
                        if jrel > 0:
                            nc.vector.memset(ex[:, 0:c0], 0.0)
                        nc.vector.tensor_mul(ex[:, c0:c0 + P], ex[:, c0:c0 + P],
                                             tri_sb[:])
                    return (ex, c0)

                def emit_pv(kt, ex, c0):
                    if kt == 0:
                        nc.vector.tensor_copy(out=acc[:], in_=ex[:])
                    else:
                        nc.vector.tensor_add(acc[:, c0:], acc[:, c0:], ex[:, c0:])
                    nc.tensor.matmul(yt_ps[:, c0:], v_sb[:, kt, h * D:(h + 1) * D],
                                     ex[:, c0:], start=(kt == 0),
                                     stop=(kt == nkt - 1))

                # 3-deep lookahead: scores for kt+1..kt+3 are issued before
                # pv(kt) so each pv's exp is ready when the in-order PE
                # reaches it
                exs = {}
                for k0 in range(min(3, nkt)):
                    exs[k0] = emit_sc(k0)
                for kt in range(nkt):
                    if kt + 3 < nkt:
                        exs[kt + 3] = emit_sc(kt + 3)
                    emit_pv(kt, *exs.pop(kt))
                row_mm(den4, h, ones_red[:], acc[:])
                yc = castpool.tile([P, TQ], BF16, name=f"yc_{h}_{qb}", tag="cast")
                nc.scalar.copy(out=yc[:], in_=yt_ps[:])
                ycs.append(yc)
                if pending_a and h == 1:
                    a_tail(*pending_a.pop(0))
            pending_a.append((qb, den4, ycs))

        # ---- output projection, one token-tile ----
        def o_pass(tt):
            for ct in range(C // P):
                o_ps = ps_tile(f"o_ps_{ct}_{tt}")
                for h in range(HG):
                    nc.tensor.matmul(o_ps[:], wo_sb[:, h, ct * P:(ct + 1) * P],
                                     yTn_sb[:, h, tt * TQ:(tt + 1) * TQ],
                                     start=(h == 0), stop=(h == HG - 1))
                o_sb = opool.tile([P, TQ], BF16, name=f"o_sb_{ct}_{tt}", tag="o_sb")
                nc.vector.tensor_copy(out=o_sb[:], in_=o_ps[:])
                nc.sync.dma_start(out=outT[ct * P:(ct + 1) * P, tt * TQ:(tt + 1) * TQ],
                                  in_=o_sb[:])

        # ---- emission: kcA as soon as gather c1 lands, kcB/v/kpe behind
        # c2, every norm tail behind filler MMs ----
        with tc.tile_pool(name="kv1pool", bufs=2) as kv1pool, \
             tc.tile_pool(name="kv2pool", bufs=2) as kv2pool, \
             tc.tile_pool(name="kcsbp", bufs=2) as kcsbp, \
             tc.tile_pool(name="qlpool", bufs=2) as qlpool:
            warm_burst(24)
            kv1s = {tt: load_lat(kv1pool, "c1", f"kv1sb{tt}", tt, 3)
                    for tt in range(2)}
            kv2s = {}
            kcps = {tt: kc_a(tt, kv1s[tt]) for tt in range(2)}
            kc_sbs = {}
            k_sqs = {}
            for tt in range(2):
                kv2s[tt] = load_lat(kv2pool, "c2", f"kv2sb{tt}", tt, 5)
                kc_sbs[tt] = kc_b(tt, kv2s[tt], kcps.pop(tt), kcsbp)
            v_pass(0, kv1s[0], kv2s[0])
            k_sqs[0] = kpe_mms(0, kv2s[0], kc_sbs.pop(0))
            kv1s.pop(0); kv2s.pop(0)
            for tt in range(2, NTT):
                kv1s[tt] = load_lat(kv1pool, "c1", f"kv1sb{tt}", tt, 3)
                kcps[tt] = kc_a(tt, kv1s[tt])
                kv2s[tt] = load_lat(kv2pool, "c2", f"kv2sb{tt}", tt, 5)
                kc_sbs[tt] = kc_b(tt, kv2s[tt], kcps.pop(tt), kcsbp)
                v_pass(tt - 1, kv1s[tt - 1], kv2s[tt - 1])
                k_sqs[tt - 1] = kpe_mms(tt - 1, kv2s[tt - 1], kc_sbs.pop(tt - 1))
                kv1s.pop(tt - 1); kv2s.pop(tt - 1)
                k_tail(tt - 2, k_sqs.pop(tt - 2))
            v_pass(NTT - 1, kv1s[NTT - 1], kv2s[NTT - 1])
            k_sqs[NTT - 1] = kpe_mms(NTT - 1, kv2s[NTT - 1], kc_sbs.pop(NTT - 1))
            k_tail(NTT - 2, k_sqs.pop(NTT - 2))
            warm_burst(12)
            k_tail(NTT - 1, k_sqs.pop(NTT - 1))

            q_state = {}
            for tt in range(NTT):
                qlsb_t = load_lat(qlpool, "q", f"qlsb{tt}", tt, NLB)
                q_state[tt] = q_mms(tt, qlsb_t)
                if tt >= 1:
                    q_tail(tt - 1, *q_state.pop(tt - 1))
                    a_block(tt - 1)
                    if tt >= 2:
                        o_pass(tt - 2)
            q_tail(NTT - 1, *q_state.pop(NTT - 1))
            a_block(NTT - 1)
            o_pass(NTT - 2)
            a_tail(*pending_a.pop(0))
            o_pass(NTT - 1)

    nc.compile()
    return nc


def _get_nc():
    if "nc" not in _NC_CACHE:
        _NC_CACHE["nc"] = build_nc()
    return _NC_CACHE["nc"]


def _prepare_in_maps(x, cos, sin, wq_a, wq_b, wkv_a, wk_b, wkpe_b, wv_b, wo):
    def bf(a):
        return np.ascontiguousarray(a).astype(NPBF16)

    cosT = np.asarray(cos, np.float32)[0, :, 0, :].T   # (64, T)
    sinT = np.asarray(sin, np.float32)[0, :, 0, :].T
    cos2 = bf(np.concatenate([cosT, cosT], axis=0))    # (128, T)
    sin2n = bf(np.concatenate([sinT, -sinT], axis=0))
    tri = (np.arange(P)[:, None] <= np.arange(P)[None, :]).astype(NPBF16)

    x = np.asarray(x, np.float32)
    wkv_a = np.asarray(wkv_a, np.float32)
    wq_a = np.asarray(wq_a, np.float32)
    # (j p) m -> p j m relayouts (partition-contiguous DRAM); kv cols
    # reordered [content 2..7, pe 0,1] so gather halves are contiguous
    wkva_r = np.concatenate([wkv_a[:, KV_PE:], wkv_a[:, :KV_PE]], axis=1)
    wkva_h = bf(wkva_r.reshape(NCC, P, LORA).transpose(1, 0, 2))
    wqa_h = bf(wq_a.reshape(NCC, P, LORA).transpose(1, 0, 2))

    def up_relay(w, k):
        return bf(np.asarray(w, np.float32).reshape(k, P, -1).transpose(1, 0, 2))

    in_maps = []
    for c in range(8):
        b, r = c // 4, c % 4
        xq = x[b, r * TQ:(r + 1) * TQ, :]                  # (512, 2048)
        xh = bf(xq.reshape(TQ, NCC, P).transpose(2, 1, 0))  # (128, 16, 512)
        hgs = slice(r * HG * D, (r + 1) * HG * D)
        wo_c = np.asarray(wo, np.float32)[hgs, :]          # (512, 2048)
        wo_h = bf(wo_c.reshape(HG, P, C).transpose(1, 0, 2))
        in_maps.append({
            "xh": xh,
            "wkva": wkva_h,
            "wqa": wqa_h,
            "wkb": up_relay(np.asarray(wk_b, np.float32)[:, hgs], NC_C),
            "wkpe": up_relay(np.asarray(wkpe_b, np.float32)[:, hgs], NC_P),
            "wvb": up_relay(np.asarray(wv_b, np.float32)[:, hgs], NC_C),
            "wqb": up_relay(np.asarray(wq_b, np.float32)[:, hgs], NLB),
            "wo": wo_h,
            "cos2": cos2,
            "sin2n": sin2n,
            "tri": tri,
        })
    return in_maps


def _assemble(results):
    out = np.empty((B, T, C), np.float32)
    for b in range(B):
        acc = results[4 * b]["outT"].astype(np.float32)
        for r in range(1, 4):
            acc = acc + results[4 * b + r]["outT"].astype(np.float32)
        out[b] = acc.T
    return out


def _run(inputs, trace=False):
    nc = _get_nc()
    in_maps = _prepare_in_maps(**inputs)
    res = run_bass_kernel_spmd(nc, in_maps, core_ids=list(range(8)), trace=trace)
    return _assemble(res.results), res


def kernel(**inputs):
    out, _ = _run(inputs)
    return out
